# revision 1
# baseline (speedup 1.0000x reference)
"""Trainium2 Bass kernel for nn_Distance_Module (retrieval_knn).

Math: out[i,j] = (dmax[i]-mn)/(mx-mn) off-diagonal, (dmin[i]-mn)/(mx-mn)
on the diagonal, where per sample i:
  s[t,f] = <text[i,t]/|..|, video[i,f]/|..|>, dmin[i] = 1-max s, dmax[i] = 1-min s,
mn = min_i dmin[i], mx = max_i dmax[i].

Device kernel (SPMD x8, batch-sharded, no collectives): per core 64 samples.
Inputs are pre-cast to bf16 on the host (the kernel is HBM-bandwidth-bound;
bf16 halves the stream and is well inside the 2e-2 tolerance).
X is shipped t-major ([T, BS, D], padded to 40 full 128-row blocks with
ones-rows) so the raw rows can be transposed/copied to the D-major store
without waiting for norms; only Y (6 blocks) is pre-normalized. Streamed in
2-block units with stage-major software pipelining (sq // transpose ->
PSUM->SBUF copy, stages lagging by tuned unit counts so every engine's
in-order queue only sees ready work); square+row-sum and copies are spread
across ACT/DVE/Pool by tuned patterns. Per-sample accumulating bf16 matmuls
(strided t-major lhsT views) -> PSUM sim matrices -> min/max reduces over F.
Because the F-reduction commutes with the positive per-row scale, X
normalization collapses to one late step: one-shot rsqrt of the norm table,
two cross-partition-offset copies interleave it into the transposed (b,t)
layout, and a single tensor-tensor multiply scales the [128,77] max/min
matrix before the final partition-range reduces -> smax/smin [128] vector.
Host: gather 8x[128], 1-s, global min/max, build the [512,512] output
(tiny).
"""

from contextlib import ExitStack

import ml_dtypes
import numpy as np

import concourse.bass as bass
import concourse.tile as tile
from concourse import masks, mybir
from concourse.bass_utils import run_bass_kernel_spmd
from concourse.vector_clock import ScopedClock

# The walrus in this toolchain only allows ONE sync-wait per instruction;
# TileContext's tail drain attaches one wait per outstanding semaphore and
# fails codegen. Split them across consecutive drains / NoOps.
_MAX_CTRL_WAITS = 1


def _split_drain_and_barrier(self, tick_clock, wait_clock):
    nc = self.nc
    drain_inst = nc.sync.drain()
    wait_clock.add_sem_waits(
        drain_inst.ins, ScopedClock({None: tick_clock.global_clock})
    )
    si = drain_inst.ins.sync_info
    waits = list(si.on_wait or []) if si else []
    if len(waits) > _MAX_CTRL_WAITS:
        si.on_wait = waits[:_MAX_CTRL_WAITS]
        for i in range(_MAX_CTRL_WAITS, len(waits), _MAX_CTRL_WAITS):
            extra = nc.sync.drain()
            esi = extra.ins.sync_info
            chunk = waits[i : i + _MAX_CTRL_WAITS]
            if esi is None:
                extra.ins.sync_info = mybir.SyncInfo(on_wait=chunk, on_update=[])
            else:
                esi.on_wait = chunk
    nc.all_engine_barrier()
    assert self.sems is not None
    popped = nc._tile_sem_poison_stack.pop()
    assert popped is self._sem_poison
    nc.clear_and_free_semaphores(list(self.sems.allocated().values()))


tile.TileContext._drain_and_barrier = _split_drain_and_barrier


def _split_sync_waits(nc, max_waits=_MAX_CTRL_WAITS):
    """Hoist extra sync-waits onto same-engine NoOps inserted just before
    the offending instruction."""
    f = nc.m.functions[0]
    for blk in f.blocks:
        out = []
        for inst in blk.instructions:
            si = getattr(inst, "sync_info", None)
            waits = list(si.on_wait) if (si and si.on_wait) else []
            if len(waits) > max_waits:
                for i in range(0, len(waits) - max_waits, max_waits):
                    nop = mybir.InstNoOp(
                        name=nc.get_next_instruction_name(), ins=[], outs=[]
                    )
                    nop.engine = inst.engine
                    nop.sync_info = mybir.SyncInfo(
                        on_wait=waits[i : i + max_waits], on_update=[]
                    )
                    nc.register_instruction(nop)
                    out.append(nop)
                si.on_wait = waits[len(waits) - max_waits :]
            out.append(inst)
        blk.instructions[:] = out


B, T, F, D = 512, 77, 12, 512
NCORES = 8
BS = B // NCORES  # 64 samples per core
XROWS = BS * T  # 4928
XPAD = 5120  # X padded to 40 full blocks with ones-rows (norms finite, unread)
YROWS = BS * F  # 768
RB = 128  # row-block partition size
NCH = D // 128  # 4 contraction chunks
GRP = 32  # samples per half / PSUM reduce group

FP32 = mybir.dt.float32
BF16 = mybir.dt.bfloat16
ALU = mybir.AluOpType
AX = mybir.AxisListType
ACTF = mybir.ActivationFunctionType

NYB = YROWS // RB  # 6
NXB = XPAD // RB  # 40 padded blocks
NBLK = NYB + NXB  # 46 NS2 columns; Y first

def _mk_pattern(n: int, weights: dict[str, float]) -> str:
    """Proportional engine assignment: at each slot pick the engine whose
    running count is furthest below its target share."""
    tot = sum(weights.values())
    counts = {e: 0 for e in weights}
    out = []
    for i in range(n):
        e = min(weights, key=lambda e: (counts[e] + 1) / (weights[e] / tot))
        counts[e] += 1
        out.append(e)
    return "".join(out)


# Engine patterns (tunable): per block for sq/scale, per copy-unit for cp.
# sq: "a"=ACT Square+accum, "d"=DVE mult+reduce, "p"=Pool mult + DVE reduce
_SQY = _mk_pattern(NBLK, {"a": 25, "p": 13, "d": 8})[:6]
SQ_PAT = _SQY + "".join(
    c * 2 for c in _mk_pattern(20, {"a": 11, "t": 5, "p": 4})
)
# sc: "d"=DVE tensor_scalar_mul (4x bf16), "a"=ACT copy-scale, "p"=Pool
SC_PAT = "dpdpdd"  # Y-only scale engines (X is scaled post-reduction)
# cp (psum->sbuf copy unit): "d"=DVE, "a"=ACT
CP_PAT = _mk_pattern(17, {"d": 13, "a": 4}) + "adadad"
SQ_LAG = 4  # units between sq(u) and its scale/transpose emission
CP_LAG = 6  # further units until the PSUM->SBUF copy emission
X_UB = 2  # X-unit size in blocks
PS_BUFS = 5  # PSUM transpose-staging buffers


def _build_body(ctx: ExitStack, tc: "tile.TileContext", text, video, dout):
    nc = tc.nc

    # Unit partition: list of (tensor, row0, nrows, ns2_col0) DMA/compute units.
    # 2-block units: Y 3 units, X 20 units (padded to 40 blocks).
    xub = X_UB
    units = []
    units.append(("y", 0, 128, 0))
    units.append(("y", 128, 128, 1))
    units.append(("y", 256, 256, 2))
    units.append(("y", 512, 256, 4))
    for u in range(40 // xub):
        units.append(("x", u * xub * RB, xub * RB, NYB + u * xub))

    nyu = 4
    grp = GRP
    n_groups = BS // grp
    # X is two t-major halves: half h's lhsT columns live in X units
    # [10h, 10h+10), so group h fires after X unit 10h+9.
    group_after_unit = {nyu + 9: [0], nyu + 19: [1]}

    const_pool = ctx.enter_context(tc.tile_pool(name="const", bufs=1))
    ident_bf = const_pool.tile([128, 128], BF16)
    ident_f32 = const_pool.tile([T, T], FP32)
    masks.make_identity(nc, ident_bf[:])
    masks.make_identity(nc, ident_f32[:])

    big_pool = ctx.enter_context(tc.tile_pool(name="big", bufs=1))
    XT = big_pool.tile([128, NCH * XPAD], BF16)
    YT = big_pool.tile([128, NCH * YROWS], BF16)
    SM = big_pool.tile([T, 2 * BS], FP32)  # cols 0:64 max-side, 64:128 min-side
    NS2 = big_pool.tile([RB, NBLK], FP32)
    RX = big_pool.tile([RB, NBLK], FP32)

    in_pool = ctx.enter_context(tc.tile_pool(name="inp", bufs=23))
    xs_pool = ctx.enter_context(tc.tile_pool(name="xsp", bufs=10))
    sq_pool = ctx.enter_context(tc.tile_pool(name="sq", bufs=6))
    inv_pool = ctx.enter_context(tc.tile_pool(name="inv", bufs=4))
    ps_pool = ctx.enter_context(
        tc.tile_pool(name="ps", bufs=PS_BUFS, space="PSUM")
    )
    g_pool = ctx.enter_context(tc.tile_pool(name="g", bufs=2, space="PSUM"))
    fin_pool = ctx.enter_context(tc.tile_pool(name="fin", bufs=1))
    smt_pool = ctx.enter_context(tc.tile_pool(name="smt", bufs=1, space="PSUM"))

    xflat = text.ap()
    yflat = video.ap()

    def emit_unit(uidx, src_kind, row0, nrows, col0):
        flat = yflat if src_kind == "y" else xflat
        dst = YT if src_kind == "y" else XT
        dst_rows = YROWS if src_kind == "y" else XROWS
        nblk = (nrows + RB - 1) // RB
        nfull = nrows // RB
        ub = in_pool.tile([RB, nblk * D], BF16, tag="xb", name=f"xb{uidx}")
        src = flat[row0 : row0 + nrows, :]
        eng = nc.sync
        if nfull:
            src_v = src[: nfull * RB, :].rearrange("(j p) d -> p j d", p=RB)
            dst_ap = ub[:].rearrange("p (j d) -> p j d", j=nblk)[:, :nfull, :]
            eng.dma_start(dst_ap, src_v)
        tail = nrows - nfull * RB
        if tail:
            eng.dma_start(
                ub[:tail, nfull * D : nfull * D + D], src[nfull * RB :, :]
            )
        return ub

    def emit_sq(uidx, ub, nrows, col0):
        nblk = (nrows + RB - 1) // RB
        # whole-unit DVE tree path: one mult + halving adds + small reduce
        if all(
            SQ_PAT[(col0 + j) % len(SQ_PAT)] == "t" for j in range(nblk)
        ):
            W = nblk * D
            sq = sq_pool.tile([RB, W], BF16, tag="sqt", name=f"sqt{uidx}")
            nc.vector.tensor_tensor(sq[:], ub[:, :W], ub[:, :W], ALU.mult)
            v = sq[:].rearrange("p (j d) -> p j d", j=nblk)
            w = D
            while w > 64:
                h = w // 2
                nc.vector.tensor_tensor(
                    v[:, :, 0:h], v[:, :, 0:h], v[:, :, h:w], ALU.add
                )
                w = h
            nc.vector.tensor_reduce(
                NS2[:, col0 : col0 + nblk], v[:, :, 0:w], axis=AX.X, op=ALU.add
            )
            return
        for j in range(nblk):
            nr = min(RB, nrows - j * RB)
            col = col0 + j
            xin = ub[:nr, j * D : (j + 1) * D]
            sq = sq_pool.tile([RB, D], BF16, tag="sq", name=f"sq{col}")
            sq_eng = SQ_PAT[col % len(SQ_PAT)]
            if sq_eng == "a":
                nc.scalar.activation(
                    sq[:nr], xin, ACTF.Square, 0.0, 1.0, 0.0,
                    accum_out=NS2[:nr, col : col + 1],
                )
            elif sq_eng == "t":
                nc.vector.tensor_tensor(sq[:nr], xin, xin, ALU.mult)
                v = sq[:nr].rearrange("p (j d) -> p j d", j=1)
                w = D
                while w > 64:
                    h = w // 2
                    nc.vector.tensor_tensor(
                        v[:, :, 0:h], v[:, :, 0:h], v[:, :, h:w], ALU.add
                    )
                    w = h
                nc.vector.tensor_reduce(
                    NS2[:nr, col : col + 1], v[:, :, 0:w], axis=AX.X, op=ALU.add
                )
            else:
                eng = nc.vector if sq_eng == "d" else nc.gpsimd
                eng.tensor_tensor(sq[:nr], xin, xin, ALU.mult)
                nc.vector.tensor_reduce(
                    NS2[:nr, col : col + 1], sq[:nr], axis=AX.X, op=ALU.add
                )

    def emit_rsqrt(col_lo, col_hi):
        for c0 in range(col_lo, col_hi, 4):
            c1 = min(c0 + 4, col_hi)
            ng = c1 - c0
            inv = inv_pool.tile([RB, 4], FP32, tag="inv", name=f"inv{c0}")
            nc.vector.reciprocal(inv[:, :ng], NS2[:, c0:c1])
            nc.scalar.sqrt(RX[:, c0:c1], inv[:, :ng])

    psts = {}

    def emit_scale_transpose(uidx, ub, src_kind, nrows, col0):
        nblk = (nrows + RB - 1) // RB
        URB = nblk * RB
        pst = ps_pool.tile([128, NCH * URB], BF16, tag="pst", name=f"pst{uidx}")
        psts[uidx] = pst
        for j in range(nblk):
            nr = min(RB, nrows - j * RB)
            col = col0 + j
            xin = ub[:nr, j * D : (j + 1) * D]
            if src_kind == "y":
                # only Y is pre-normalized; X rows are scaled post-reduction
                rx = RX[:nr, col : col + 1]
                xs = xs_pool.tile([RB, D], BF16, tag="xs", name=f"xs{col}")
                sc_eng = SC_PAT[col % len(SC_PAT)]
                if sc_eng == "d":
                    nc.vector.tensor_scalar_mul(xs[:nr], xin, rx)
                elif sc_eng == "a":
                    nc.scalar.mul(xs[:nr], xin, rx)
                else:
                    nc.gpsimd.tensor_scalar_mul(xs[:nr], xin, rx)
                xin = xs[:nr]
            for c in range(NCH):
                nc.tensor.transpose(
                    pst[:, c * URB + j * RB : c * URB + j * RB + nr],
                    xin[:, c * RB : (c + 1) * RB],
                    ident_bf[:nr, :nr],
                )

    def emit_copy(uidx, src_kind, row0, nrows):
        dst = YT if src_kind == "y" else XT
        pst = psts.pop(uidx)
        nblk = (nrows + RB - 1) // RB
        URB = nblk * RB
        dst_v = dst[:].rearrange("p (c r) -> p c r", c=NCH)
        pst_v = pst[:].rearrange("p (c r) -> p c r", c=NCH)
        srcp = pst_v[:, :, :nrows]
        out_ap = dst_v[:, :, row0 : row0 + nrows]
        cp_eng = CP_PAT[uidx % len(CP_PAT)]
        if cp_eng == "d":
            nc.vector.tensor_copy(out_ap, srcp)
        elif cp_eng == "a":
            nc.scalar.copy(out_ap, srcp)
        else:
            nc.gpsimd.tensor_copy(out_ap, srcp)

    def emit_sim_group(g):
        """Similarity matmuls + min/max reduces for grp samples."""
        gps = g_pool.tile([T, grp * F], FP32, tag="g", name=f"g{g}")
        XTv = XT[:].rearrange("p (c h t b) -> p c h t b", c=NCH, h=2, b=32)
        for j in range(grp):
            b = g * grp + j
            for c in range(NCH):
                nc.tensor.matmul(
                    gps[:, j * F : (j + 1) * F],
                    XTv[:, c, g, 0:T, j],
                    YT[:, c * YROWS + b * F : c * YROWS + (b + 1) * F],
                    start=(c == 0),
                    stop=(c == NCH - 1),
                )
        gv = gps[:].rearrange("p (j f) -> p j f", f=F)
        nc.vector.tensor_reduce(
            SM[:, g * grp : (g + 1) * grp], gv, axis=AX.X, op=ALU.max
        )
        nc.vector.tensor_reduce(
            SM[:, BS + g * grp : BS + (g + 1) * grp], gv, axis=AX.X, op=ALU.min
        )

    # issue every input DMA up front; the unit-deep tile pool never recycles
    ubs = []
    for uidx, (kind, row0, nrows, col0) in enumerate(units):
        ubs.append(emit_unit(uidx, kind, row0, nrows, col0))

    RX2 = fin_pool.tile([2 * BS, NXB * 2], FP32)
    RX2v = RX2[:].rearrange("p (t e) -> p t e", e=4)

    def emit_norm_all():
        """rsqrt + (b,t)-interleave + negated min-side rows for X."""
        emit_rsqrt(NYB, NBLK)
        for h in range(2):
            for r in range(4):
                nc.vector.tensor_copy(
                    RX2v[32 * h : 32 * h + 32, :, r],
                    RX[32 * r : 32 * r + 32, NYB + 20 * h : NYB + 20 * h + 20],
                )
        nc.vector.tensor_scalar(
            RX2[BS : 2 * BS, :], RX2[0:BS, :], -1.0, 0.0, ALU.mult, ALU.add
        )

    # Stage-major software pipelining: each engine's in-order program sees
    # only instructions whose cross-engine deps resolved units ago.
    #   sq(u) at step u; rsqrt pair k after sq(2k+3); scale/transpose pair k
    #   right after; copies lag one pair; groups fire as their last X copy
    #   lands.
    n_units = len(units)
    st_done = 0
    cp_done = 0
    rs_col = 0

    def flush(upto_st, upto_cp):
        nonlocal st_done, cp_done, rs_col
        if upto_st > st_done and units[upto_st - 1][0] == "y":
            kind, row0, nrows, col0 = units[upto_st - 1]
            col_need = col0 + (nrows + RB - 1) // RB
            if col_need > rs_col:
                emit_rsqrt(rs_col, min(col_need + 2, NYB))
                rs_col = min(col_need + 2, NYB)
        while st_done < upto_st:
            kind, row0, nrows, col0 = units[st_done]
            emit_scale_transpose(st_done, ubs[st_done], kind, nrows, col0)
            st_done += 1
        while cp_done < upto_cp:
            kind, row0, nrows, col0 = units[cp_done]
            emit_copy(cp_done, kind, row0, nrows)
            cp_done += 1
            for g in group_after_unit.get(cp_done - 1, ()):
                emit_sim_group(g)

    sq_lag, cp_lag = SQ_LAG, CP_LAG
    for uidx in range(n_units):
        kind, row0, nrows, col0 = units[uidx]
        flush(max(0, uidx + 1 - sq_lag), max(0, uidx + 1 - sq_lag - cp_lag))
        emit_sq(uidx, ubs[uidx], nrows, col0)
    flush(n_units, n_units)

    # finals: one-shot X rsqrt; rx[t,b] rebuilt into smt layout with two
    # cross-partition-offset copies (even t rows sit on partitions 0:64 of
    # RX, odd t on 64:128); scale the transposed max/min matrix, then
    # partition-range reduces and one DMA out.
    emit_norm_all()
    smt = smt_pool.tile([2 * BS, T], FP32, tag="smt")
    nc.tensor.transpose(smt[:, :], SM[:, :], ident_f32[:, :])
    nc.vector.tensor_tensor(smt[:, :], smt[:, :], RX2[:, 0:T], ALU.mult)
    douts = fin_pool.tile([2 * BS, 1], FP32)
    nc.vector.tensor_reduce(douts[:, 0:1], smt[:, :], axis=AX.X, op=ALU.max)
    nc.sync.dma_start(dout.ap().rearrange("(p c) -> p c", c=1), douts[:, :])


def build():
    nc = bass.Bass("TRN2", target_bir_lowering=False, debug=False)
    text = nc.dram_tensor("text", [XPAD, D], BF16, kind="ExternalInput")
    video = nc.dram_tensor("video", [YROWS, D], BF16, kind="ExternalInput")
    dout = nc.dram_tensor("dout", [2 * BS], FP32, kind="ExternalOutput")
    with tile.TileContext(nc) as tc:
        with ExitStack() as ctx:
            _build_body(ctx, tc, text, video, dout)
    _split_sync_waits(nc)
    return nc


_nc_cache = None


def _get_nc():
    global _nc_cache
    if _nc_cache is None:
        _nc_cache = build()
    return _nc_cache


def run_device(text: np.ndarray, video: np.ndarray, trace: bool = False):
    """Run the SPMD kernel on 8 cores; returns (smax[B], smin[B], results)."""
    nc = _get_nc()
    bf = ml_dtypes.bfloat16
    in_maps = [
        {
            "text": np.ascontiguousarray(
                np.concatenate(
                    [
                        arr
                        for h in range(2)
                        for arr in (
                            text[i * BS + h * 32 : i * BS + (h + 1) * 32]
                            .transpose(1, 0, 2)
                            .reshape(32 * T, D),
                            np.ones((2560 - 32 * T, D), dtype=np.float32),
                        )
                    ]
                ).astype(bf)
            ),
            "video": np.ascontiguousarray(
                video[i * BS : (i + 1) * BS].reshape(YROWS, D).astype(bf)
            ),
        }
        for i in range(NCORES)
    ]
    res = run_bass_kernel_spmd(nc, in_maps, list(range(NCORES)), trace=trace)
    douts = [
        np.asarray(res.results[i]["dout"], dtype=np.float32) for i in range(NCORES)
    ]
    smax = np.concatenate([d[:BS] for d in douts])
    smin = np.concatenate([-d[BS:] for d in douts])
    return smax, smin, res


def kernel(Prob_text: np.ndarray, Prob_video: np.ndarray) -> np.ndarray:
    text = np.ascontiguousarray(np.asarray(Prob_text, dtype=np.float32))
    video = np.ascontiguousarray(np.asarray(Prob_video, dtype=np.float32))
    smax, smin, _ = run_device(text, video)
    dmin = 1.0 - smax
    dmax = 1.0 - smin
    mn = dmin.min()
    mx = dmax.max()
    dis = np.broadcast_to(dmax[:, None], (B, B)).copy()
    np.fill_diagonal(dis, dmin)
    return ((dis - mn) / (mx - mn)).astype(np.float32)

